# revision 38
# baseline (speedup 1.0000x reference)
"""Trainium2 Bass kernel for nn_ContinuousEmbedding (masked matmul + bias).

Computes out = x @ (weights * mask) + bias, reshaped to [B, in_size, out_size],
where mask zeroes each input feature's own [out_size]-wide diagonal block.

Strategy: tensor-parallel across the 8 NeuronCores by splitting the
in_size*out_size (=16384) output columns into 8 shards of 2048 columns.
The rel-err budget (2e-2) is large, so all matmul I/O is bf16: inputs are
cast on the host, the PE runs bf16 at full rate, and the output shard is
stored to HBM as bf16 (halving the dominant store traffic) then upcast on
the host.

Compute orientation is TRANSPOSED vs the torch view: each core computes
out_t[col, batch], i.e. matmul with lhsT = W[k, col_block] (stationary)
and rhs = x^T[k, batch] (moving).  That puts the io-columns on PSUM
partitions, so the bias becomes a per-partition scalar — eviction is a
1-op fused add+cast via tensor_scalar (DVE) / activation-Identity (ACT),
alternating between the two engines so eviction keeps up with the PE.
The host transposes the gathered [2048, 4096] shards back to [B, io].

All inputs (bias, masked W shard, x^T) are packed on the host into ONE
[128, 12304] bf16 "blob" in consumption order, loaded by 5 fat DMA
chunks (early delivery degrades with chunk count).  The first chunk is
only the k0 half of what col-blocks 0-3 need (~400 KB), so the real
stream starts as soon as it lands; the k1 half arrives one chunk later
with PE fill matmuls bridging the k0->k1 seam of the first pairs.

A dense PE warm-up runs from the engine prologue until the first chunk
lands: the HAM clock-gate needs ~3.5us of gapless PE activity to ramp
to 8/8, and a PE gap before that resets the ramp (half-clock stream).

Tail: the last two col-blocks store per group, and the last two groups
evict each 512-wide half as soon as its stop-matmul retires, split
across both engines, with per-half stores at the very end so the last
bytes leave ASAP.

Mask is constant — folded into the weights on the host.
"""

import numpy as np

B = 4096
IN_SIZE = 256
OUT_SIZE = 64
IO = IN_SIZE * OUT_SIZE          # 16384
N_CORES = 8
N_SHARD = IO // N_CORES          # 2048 output columns per core
P = 128                          # SBUF/PSUM partitions
KO = IN_SIZE // P                # 2 contraction sub-tiles
M_BLOCKS = N_SHARD // P          # 16 col-blocks per core
N_TILE = 512                     # matmul moving free dim (fp32 PSUM bank)
G_TILE = 1024                    # eviction group width (2 PSUM banks)
G_PER_M = B // G_TILE            # 4 groups per col-block
PSUM_BUFS = 4                    # 4 x 2 banks = all 8 PSUM banks
INTER = 4                        # col-blocks processed group-major first
WARM_MM = 40                     # PE warm-up matmuls: dense activity until
                                 # the first chunk lands (HAM clock ramp)

# ---- blob column layout (bf16 elements, consumption order) ----
OFF_BIAS = 0                                   # 16
OFF_W0K0 = OFF_BIAS + M_BLOCKS                 # 16: W k0 m0..3
OFF_XT_G0K0 = OFF_W0K0 + INTER * P             # 528: xt k0 g0
OFF_W0K1 = OFF_XT_G0K0 + G_TILE                # 1552: W k1 m0..3
OFF_XT_G0K1 = OFF_W0K1 + INTER * P             # 2064: xt k1 g0
OFF_XT_G1 = OFF_XT_G0K1 + G_TILE               # 3088: xt g1 (k0,k1)
OFF_XT_G2 = OFF_XT_G1 + KO * G_TILE            # 5136: xt g2
OFF_XT_G3 = OFF_XT_G2 + KO * G_TILE            # 7184: xt g3
OFF_W4 = OFF_XT_G3 + KO * G_TILE               # 9232: W m4..15
TOTAL = OFF_W4 + (M_BLOCKS - INTER) * KO * P   # 12304

# Few FAT chunks (early delivery degrades with chunk count); chunk 0 is
# the k0 half of everything col-blocks 0-3 consume.
CHUNKS = [
    (0, OFF_W0K1),                             # bias + W m0..3 k0 + xt g0 k0
    (OFF_W0K1, OFF_XT_G1),                     # W m0..3 k1 + xt g0 k1
    (OFF_XT_G1, OFF_XT_G2),                    # xt g1
    (OFF_XT_G2, OFF_W4),                       # xt g2 + g3
    (OFF_W4, TOTAL),                           # W m4..15
]

# Clock-keeper fills (PE dummy matmuls) bridging the k0->k1 seam of the
# first pairs while chunk 1 lands: a PE stall before the HAM ramp fires
# resets it, so fill instead of stalling.  Keyed by (pos, k) emission
# point.  (pos 3 would alias warm_ps's PSUM buffer — keep to pos 0-2.)
FILLS = {}                       # k-blocked prologue replaced the fills


def _w_off(k, m):
    if m < INTER:
        return (OFF_W0K0 if k == 0 else OFF_W0K1) + m * P
    return OFF_W4 + (m - INTER) * KO * P + k * P


def _xt_off(k, n):
    g, r = divmod(n, G_TILE)
    if g == 0:
        return (OFF_XT_G0K0 if k == 0 else OFF_XT_G0K1) + r
    base = {1: OFF_XT_G1, 2: OFF_XT_G2, 3: OFF_XT_G3}[g]
    return base + k * G_TILE + r


_CACHE: dict = {}


def _build_program():
    import concourse.mybir as mybir
    import concourse.tile as tile
    from concourse import bacc

    nc = bacc.Bacc(
        "TRN2", target_bir_lowering=False, debug=False, num_devices=N_CORES
    )
    bf16 = mybir.dt.bfloat16
    f32 = mybir.dt.float32
    blob = nc.dram_tensor("blob", [P, TOTAL], bf16, kind="ExternalInput").ap()
    # transposed output shard: out_t[col, batch]
    out = nc.dram_tensor("out", [N_SHARD, B], bf16, kind="ExternalOutput").ap()

    with tile.TileContext(nc) as tc:
        with tc.tile_pool(name="const", bufs=1) as const, \
             tc.tile_pool(name="psum", bufs=PSUM_BUFS, space="PSUM") as psum_pool, \
             tc.tile_pool(name="outp", bufs=6) as outp:
            blob_sb = const.tile([P, TOTAL], bf16)

            # Loads in consumption order on a single HWDGE ring (the
            # second ring is heavily throttled — do not split).
            ld = nc.sync
            for lo, hi in CHUNKS:
                ld.dma_start(out=blob_sb[:, lo:hi], in_=blob[:, lo:hi])

            # Warm-up while inputs stream in; a dummy activation pulls
            # the ACT function table in early.
            warm_w = const.tile([P, P], bf16)
            warmf = const.tile([1, 1], f32)
            nc.vector.memset(warm_w, 0.0)
            nc.vector.memset(warmf, 0.0)
            nc.scalar.add(warmf[:], warmf[:], warmf[0:1, 0:1])
            # Unpack the packed bf16 bias columns to f32 (DVE scalar
            # operands must be f32).
            bias_sb = const.tile([P, M_BLOCKS], f32)
            nc.vector.tensor_copy(bias_sb[:], blob_sb[:, 0:M_BLOCKS])
            warm_ps = psum_pool.tile([P, G_TILE], f32, name="warm_ps", tag="ps")
            for _ in range(WARM_MM):
                nc.tensor.matmul(
                    warm_ps[:, 0:P], lhsT=warm_w[:], rhs=warm_w[:],
                    start=True, stop=True,
                )

            # Execution order: group-major over the first INTER col-blocks
            # (so full x^T is only needed after ~16 groups), then
            # block-major for the rest.
            order = [(m, g) for g in range(G_PER_M) for m in range(INTER)]
            order += [(m, g) for m in range(INTER, M_BLOCKS)
                      for g in range(G_PER_M)]
            out_sbs = {}

            # First four pairs (col-blocks 0-3, group 0) run k-blocked:
            # all eight k0 matmuls (which need only chunk 0) before any
            # k1 matmul, so chunk 1 has ~1.7us of real work to hide
            # behind instead of a fill-bridged seam.  All four PSUM
            # tiles accumulate simultaneously (8 banks).
            g0_ps = {}
            for m in range(INTER):
                out_sbs[m] = outp.tile([P, B], bf16, name=f"osb{m}",
                                       tag="osb")
                g0_ps[m] = psum_pool.tile([P, G_TILE], f32, name=f"ps{m}_0",
                                          tag="ps")
            for k in range(KO):
                for m in range(INTER):
                    wof = _w_off(k, m)
                    for s in range(G_TILE // N_TILE):
                        xof = _xt_off(k, s * N_TILE)
                        nc.tensor.matmul(
                            g0_ps[m][:, s * N_TILE:(s + 1) * N_TILE],
                            lhsT=blob_sb[:, wof:wof + P],
                            rhs=blob_sb[:, xof:xof + N_TILE],
                            start=(k == 0),
                            stop=(k == KO - 1),
                        )
            for m in range(INTER):
                gs = slice(0, G_TILE)
                if m % 2 == 0:
                    nc.vector.tensor_scalar_add(
                        out_sbs[m][:, gs], g0_ps[m][:], bias_sb[:, m:m + 1]
                    )
                else:
                    nc.scalar.add(out_sbs[m][:, gs], g0_ps[m][:],
                                  bias_sb[:, m:m + 1])

            for pos, (m, g) in enumerate(order):
                if pos < INTER:
                    continue  # handled by the k-blocked prologue above
                ms = slice(m * P, (m + 1) * P)
                if m not in out_sbs:
                    out_sbs[m] = outp.tile([P, B], bf16, name=f"osb{m}",
                                           tag="osb")
                out_sb = out_sbs[m]
                ps = psum_pool.tile([P, G_TILE], f32, name=f"ps{m}_{g}",
                                    tag="ps")
                for k in range(KO):
                    for _ in range(FILLS.get((pos, k), 0)):
                        nc.tensor.matmul(
                            warm_ps[:, 0:P], lhsT=warm_w[:], rhs=warm_w[:],
                            start=True, stop=True,
                        )
                    wof = _w_off(k, m)
                    for s in range(G_TILE // N_TILE):
                        n0 = g * G_TILE + s * N_TILE
                        xof = _xt_off(k, n0)
                        nc.tensor.matmul(
                            ps[:, s * N_TILE:(s + 1) * N_TILE],
                            lhsT=blob_sb[:, wof:wof + P],
                            rhs=blob_sb[:, xof:xof + N_TILE],
                            start=(k == 0),
                            stop=(k == KO - 1),
                        )
                gs = slice(g * G_TILE, (g + 1) * G_TILE)
                if m == M_BLOCKS - 1:
                    # Last col-block: evict each 512-wide half as soon
                    # as its stop-matmul retires (s0 stops one matmul
                    # before s1), split across both engines so neither
                    # backlogs, but store each group as ONE dma — the
                    # ~0.6us issue per dma_start serializes on Sync, so
                    # fewer issues beat earlier halves at the very end.
                    # Bias for the end zone is added on the HOST: a pure
                    # cast eviction may beat tensor_scalar's rate, and
                    # the end zone is eviction-duty-bound.
                    h = G_TILE // 2
                    nc.vector.tensor_copy(
                        out_sb[:, gs.start:gs.start + h], ps[:, 0:h]
                    )
                    nc.scalar.add(
                        out_sb[:, gs.start + h:gs.stop], ps[:, h:], 0.0
                    )
                    ld.dma_start(out=out[ms, gs], in_=out_sb[:, gs])
                    continue
                if m == M_BLOCKS - 2:
                    # End zone col-block 14: bias-free cast evictions
                    # (host adds bias), per-group stores.
                    if pos % 2 == 0:
                        nc.vector.tensor_copy(out_sb[:, gs], ps[:])
                    else:
                        nc.scalar.add(out_sb[:, gs], ps[:], 0.0)
                    ld.dma_start(out=out[ms, gs], in_=out_sb[:, gs])
                    continue
                if pos % 2 == 0:
                    nc.vector.tensor_scalar_add(
                        out_sb[:, gs], ps[:], bias_sb[:, m:m + 1]
                    )
                else:
                    nc.scalar.add(out_sb[:, gs], ps[:], bias_sb[:, m:m + 1])
                if g % 2 == 1:
                    # Stores per batch-half (512 KB each, smooth stream).
                    hs = slice((g - 1) * G_TILE, (g + 1) * G_TILE)
                    ld.dma_start(out=out[ms, hs], in_=out_sb[:, hs])

    nc.compile()
    return nc


def _get_program():
    if "prog" not in _CACHE:
        _CACHE["prog"] = _build_program()
    return _CACHE["prog"]


def _shard_inputs(x, weights, bias):
    import ml_dtypes

    bf16 = ml_dtypes.bfloat16
    # Fold the constant block-diagonal mask into the weights on the host.
    col_block = np.arange(IO, dtype=np.int64) // OUT_SIZE
    mask = (col_block[None, :] != np.arange(IN_SIZE)[:, None])
    wm = (weights * mask.astype(weights.dtype)).astype(bf16)
    xt = np.ascontiguousarray(x.T.astype(bf16))
    in_maps = []
    for c in range(N_CORES):
        sl = slice(c * N_SHARD, (c + 1) * N_SHARD)
        ws = wm[:, sl]
        blob = np.empty((P, TOTAL), dtype=bf16)
        blob[:, OFF_BIAS:OFF_BIAS + M_BLOCKS] = \
            bias[sl].reshape(M_BLOCKS, P).T.astype(bf16)
        for k in range(KO):
            kr = slice(k * P, (k + 1) * P)
            for m in range(M_BLOCKS):
                blob[:, _w_off(k, m):_w_off(k, m) + P] = \
                    ws[kr, m * P:(m + 1) * P]
            for g in range(G_PER_M):
                blob[:, _xt_off(k, g * G_TILE):_xt_off(k, g * G_TILE) +
                     G_TILE] = xt[kr, g * G_TILE:(g + 1) * G_TILE]
        in_maps.append({"blob": blob})
    return in_maps


def run_sharded(in_maps, **kwargs):
    """Run the SPMD program on cores 0-7. kwargs forwarded (e.g. trace)."""
    from concourse.bass_utils import run_bass_kernel_spmd

    nc = _get_program()
    return run_bass_kernel_spmd(
        nc, in_maps, core_ids=list(range(N_CORES)), **kwargs
    )


def kernel(x: np.ndarray, weights: np.ndarray, bias: np.ndarray) -> np.ndarray:
    x = np.asarray(x, dtype=np.float32)
    weights = np.asarray(weights, dtype=np.float32)
    bias = np.asarray(bias, dtype=np.float32)
    in_maps = _shard_inputs(x, weights, bias)
    res = run_sharded(in_maps)
    full = np.empty((B, IO), dtype=np.float32)
    for c in range(N_CORES):
        sl = slice(c * N_SHARD, (c + 1) * N_SHARD)
        full[:, sl] = np.asarray(res.results[c]["out"]).astype(np.float32).T
        # Device skips the bias add for the end-zone col-blocks (14, 15)
        # so their evictions are pure casts; add it here instead.
        ez = slice(c * N_SHARD + (M_BLOCKS - 2) * P, (c + 1) * N_SHARD)
        full[:, ez] += bias[ez]
    return full.reshape(B, IN_SIZE, OUT_SIZE)
